# revision 18
# baseline (speedup 1.0000x reference)
"""Trainium2 Bass kernel for the octonion causal self-attention block.

Strategy (8 NeuronCores, SPMD):
  Each core owns one octonion component c (= heads 2c, 2c+1).
  - Host prep: ternary-quantize weights exactly as the reference does and
    keep them as EXACT {-1,0,+1} matrices (fp8/bf16 representable); the
    scalar scales are folded into (a) the exp() activation scale for
    s_q*s_k/sqrt(HD), and (b) the host-side unshard for s_v*s_o.
    Assemble the effective [C, C] block matrices (sign/permutation
    combine folded in), permute q/k output channels into RoPE
    split-layout, and transpose x to channel-major xT in chunk-
    contiguous layout (one fp8 copy for q/k, one bf16 copy for v).
  - Device phase 0: a short burst of warm-up matmuls on a zero tile so
    the PE HAM clock-gate reaches K=8/8 during the initial DMA wait.
  - Device phase 1: q/k projections in fp8 DoubleRow (2x contraction per
    cycle; ternary weights are exact in fp8, only x carries quantization
    noise which the softmax largely cancels), v projection in bf16.
    RoPE applied on the fly; qT/kT kept channel-major in SBUF.
  - Device phase 2: causal attention per (batch, head): S^T = K^T Q per
    s-tile, exp with the folded score scale (no max subtraction --
    scores are O(1) bounded), triangular mask on the diagonal tile,
    P^T V with an appended ones-column in V giving the softmax
    denominator for free, per-partition normalize, PE-transpose back to
    channel-major yT (kept in SBUF).
  - Device phase 3: PARTIAL output projection with ternary bf16 Wo:
    out_partial^T = Wo[rows of component c]^T @ yT_c -- no collective.
    The host scales by s_v*s_o and sums the 8 partial outputs while
    unsharding.
"""

import numpy as np
import ml_dtypes

import concourse.bass as bass
import concourse.tile as tile
from concourse import bacc, mybir
from concourse.bass_utils import run_bass_kernel_spmd
from concourse.masks import make_identity

# ---------------------------------------------------------------- problem dims
B, T_FULL, C, H = 2, 2048, 2048, 16
HD = C // H          # 128
P = C // 8           # 256
N_CORES = 8
KT = C // 128        # 16 contraction k-tiles

OCT_SIGN = np.array([
    [1, 1, 1, 1, 1, 1, 1, 1],
    [1,-1, 1,-1, 1,-1,-1, 1],
    [1,-1,-1, 1, 1, 1,-1,-1],
    [1, 1,-1,-1, 1,-1, 1,-1],
    [1,-1,-1,-1,-1, 1, 1, 1],
    [1, 1,-1, 1,-1,-1,-1, 1],
    [1, 1, 1,-1,-1, 1,-1,-1],
    [1,-1, 1, 1,-1,-1, 1,-1]], dtype=np.float32)
OCT_IDX = np.array([
    [0,1,2,3,4,5,6,7],
    [1,0,3,2,5,4,7,6],
    [2,3,0,1,6,7,4,5],
    [3,2,1,0,7,6,5,4],
    [4,5,6,7,0,1,2,3],
    [5,4,7,6,1,0,3,2],
    [6,7,4,5,2,3,0,1],
    [7,6,5,4,3,2,1,0]], dtype=np.int32)
_COMB = np.zeros((8, 8, 8), np.float32)
for _i in range(8):
    for _j in range(8):
        _COMB[OCT_IDX[_i, _j], _i, _j] = OCT_SIGN[_i, _j]

BF16 = ml_dtypes.bfloat16
FP8 = ml_dtypes.float8_e4m3


# ------------------------------------------------------------------- host prep
def _ternary(W: np.ndarray) -> tuple[np.ndarray, np.float32]:
    """Reference's ternary quantization, returned as exact {-1,0,1} + scale."""
    try:
        import jax
        import jax.numpy as jnp
        cpu = jax.local_devices(backend="cpu")[0]
        with jax.default_device(cpu):
            Wj = jnp.asarray(W)
            s = jnp.mean(jnp.abs(Wj)) + 1e-8
            t = jnp.round(jnp.clip(Wj / s, -1.0, 1.0))
            return np.asarray(t, np.float32), np.float32(s)
    except Exception:
        s = np.float32(np.mean(np.abs(W.astype(np.float32)))) + np.float32(1e-8)
        return np.rint(np.clip(W / s, -1.0, 1.0)).astype(np.float32), s


def _build_t_eff(W: np.ndarray) -> tuple[np.ndarray, np.float32]:
    """[8,P,P] weights -> exact ternary effective [C, C] + scale."""
    Tq, s = _ternary(W)  # (8, P, P) in {-1,0,1}
    # T_eff[(j,p),(k,q)] = sum_i COMB[k,i,j] * Tq[i,p,q]; exactly one i per (j,k)
    return (np.einsum("kij,ipq->jpkq", _COMB, Tq).reshape(C, C), s)


def _rope_colperm() -> np.ndarray:
    """colperm[new] = old: within each head, [re0..re63 | im0..im63]."""
    perm = np.zeros(C, dtype=np.int64)
    for h in range(H):
        base = h * HD
        for r in range(HD // 2):
            perm[base + r] = base + 2 * r
            perm[base + HD // 2 + r] = base + 2 * r + 1
    return perm


def prep_inputs(inputs: dict, T: int) -> list[dict]:
    """Build the 8 per-core input maps from the full problem inputs."""
    NT = B * T
    TCH = min(512, T)
    NCH = NT // TCH
    x = np.asarray(inputs["x"], np.float32)[:, :T, :]
    cos = np.asarray(inputs["freqs_cos"], np.float32)[:T]   # [T, 64]
    sin = np.asarray(inputs["freqs_sin"], np.float32)[:T]

    tq_eff, s_q = _build_t_eff(np.asarray(inputs["wq"], np.float32))
    tk_eff, s_k = _build_t_eff(np.asarray(inputs["wk"], np.float32))
    tv_eff, s_v = _build_t_eff(np.asarray(inputs["wv"], np.float32))
    to_eff, s_o = _build_t_eff(np.asarray(inputs["wo"], np.float32))

    perm = _rope_colperm()
    tq_eff = tq_eff[:, perm]
    tk_eff = tk_eff[:, perm]

    # xT [C, NT] in chunk-contiguous layout [NCH, 128, KT, TCH]:
    # element [ch, p, k, t] = xT[k*128+p, ch*TCH+t] -> per-partition lines
    # are KT*TCH contiguous elements (one DMA descriptor per partition).
    xt = x.reshape(NT, C).T.reshape(KT, 128, NCH, TCH).transpose(2, 1, 0, 3)
    xt8 = np.ascontiguousarray(xt.astype(FP8))
    xtb = np.ascontiguousarray(xt.astype(BF16))

    # rope tables, duplicated-half layout [128, T]
    cosd = np.empty((128, T), np.float32)
    cosd[0:64] = cos.T
    cosd[64:128] = cos.T
    sind = np.empty((128, T), np.float32)
    sind[0:64] = -sin.T
    sind[64:128] = sin.T
    cosd = cosd.astype(BF16)
    sind = sind.astype(BF16)

    tri = np.triu(np.ones((128, 128), np.float32)).astype(BF16)  # [s,q] s<=q

    # folded score scale for the exp() activation
    alpha = np.full((128, 1), s_q * s_k * (HD ** -0.5), np.float32)
    out_scale = float(s_v) * float(s_o)

    def wblocks(t_eff: np.ndarray, c: int, dt) -> np.ndarray:
        # [C, 256] block -> [128, KT, 256] (partition-major, contiguous lines)
        blk = t_eff[:, c * P:(c + 1) * P].reshape(KT, 128, P).transpose(1, 0, 2)
        return np.ascontiguousarray(blk.astype(dt))

    in_maps = []
    for c in range(N_CORES):
        # o-proj row-block for component c: [256, C] -> [2, 128, C]
        wo_rows = np.ascontiguousarray(
            to_eff[c * P:(c + 1) * P, :].reshape(2, 128, C).astype(BF16))
        in_maps.append({
            "xt8": xt8,
            "xtb": xtb,
            "wq": wblocks(tq_eff, c, FP8),
            "wk": wblocks(tk_eff, c, FP8),
            "wv": wblocks(tv_eff, c, BF16),
            "wo": wo_rows,
            "cosd": cosd,
            "sind": sind,
            "tri": tri,
            "alpha": alpha,
        })
    return in_maps, out_scale


# ------------------------------------------------------------- device program
def build_nc(T: int = T_FULL, n_cores: int = N_CORES):
    NT = B * T
    ST = T // 128            # s-tiles per batch
    NST = NT // 128
    TCH = min(512, T)        # token chunk; must not cross a batch boundary
    NCH = NT // TCH
    bf16 = mybir.dt.bfloat16
    fp8 = mybir.dt.float8e4
    f32 = mybir.dt.float32
    DR = mybir.MatmulPerfMode.DoubleRow

    nc = bacc.Bacc("TRN2", target_bir_lowering=False, debug=False,
                   num_devices=n_cores)

    xt8_d = nc.dram_tensor("xt8", [NCH, 128, KT, TCH], fp8, kind="ExternalInput")
    xtb_d = nc.dram_tensor("xtb", [NCH, 128, KT, TCH], bf16, kind="ExternalInput")
    wq_d = nc.dram_tensor("wq", [128, KT, P], fp8, kind="ExternalInput")
    wk_d = nc.dram_tensor("wk", [128, KT, P], fp8, kind="ExternalInput")
    wv_d = nc.dram_tensor("wv", [128, KT, P], bf16, kind="ExternalInput")
    wo_d = nc.dram_tensor("wo", [2, 128, C], bf16, kind="ExternalInput")
    cos_d = nc.dram_tensor("cosd", [128, T], bf16, kind="ExternalInput")
    sin_d = nc.dram_tensor("sind", [128, T], bf16, kind="ExternalInput")
    tri_d = nc.dram_tensor("tri", [128, 128], bf16, kind="ExternalInput")
    alpha_d = nc.dram_tensor("alpha", [128, 1], f32, kind="ExternalInput")
    out_d = nc.dram_tensor("outt", [C, NT], bf16, kind="ExternalOutput")

    with tile.TileContext(nc) as tc:
        # One flat pool scope: proj- and attention-phase pools coexist in
        # disjoint SBUF/PSUM, so the scheduler can overlap batch-0 attention
        # (whose ACT exp stream is the kernel bottleneck) with the tail of
        # the projection phase instead of serializing on pool reuse.
        with (
            tc.tile_pool(name="consts", bufs=1) as consts,
            tc.tile_pool(name="persist", bufs=1) as persist,
            tc.tile_pool(name="xt8s", bufs=2) as xt8_pool,
            tc.tile_pool(name="xtbs", bufs=2) as xtb_pool,
            tc.tile_pool(name="rope", bufs=2) as rope_pool,
            tc.tile_pool(name="pt", bufs=1) as pt_pool,
            tc.tile_pool(name="att_small", bufs=4) as small_pool,
            tc.tile_pool(name="ysb", bufs=ST) as ysb_pool,
            tc.tile_pool(name="ystg", bufs=1) as ystg_pool,
            tc.tile_pool(name="ostage", bufs=6) as o_pool,
            tc.tile_pool(name="psbig", bufs=3, space="PSUM") as psbig,
            tc.tile_pool(name="psv", bufs=2, space="PSUM") as psv,
            tc.tile_pool(name="ps_y", bufs=2, space="PSUM") as ps_y,
            tc.tile_pool(name="ps_t", bufs=1, space="PSUM") as ps_t,
        ):
            # ---- PE warm-up: ~3.5us of junk matmuls with no DMA deps so
            # the HAM clock-gate flips to K=8/8 while inputs stream in.
            warm = consts.tile([128, 512], bf16, tag="warm")
            nc.vector.memset(warm[:], 0.0)
            ps_w = psbig.tile([128, 512], f32, tag="big")
            for _ in range(8):
                nc.tensor.matmul(ps_w[:], lhsT=warm[:, 0:128], rhs=warm[:],
                                 start=True, stop=True)

            # ---- resident constants (DMA order = PE consumption order;
            # rope tables come AFTER the chunk-0/1 activations since only
            # the DVE rope needs them, and tri/wo are deferred into the
            # chunk loop -- they are first read in the attention phase)
            alpha_s = consts.tile([128, 1], f32, tag="alpha")
            nc.sync.dma_start(out=alpha_s, in_=alpha_d.ap())
            wq_s = consts.tile([128, KT, P], fp8, tag="wq")
            wk_s = consts.tile([128, KT, P], fp8, tag="wk")
            wv_s = consts.tile([128, KT, P], bf16, tag="wv")
            wo_s = consts.tile([128, 2, C], bf16, tag="wo")
            xt8_0 = xt8_pool.tile([128, KT, TCH], fp8, tag="xt8")
            for kq in range(0, KT, 4):
                nc.sync.dma_start(out=wq_s[:, kq:kq + 4, :],
                                  in_=wq_d.ap()[:, kq:kq + 4, :])
                nc.sync.dma_start(out=xt8_0[:, kq:kq + 4, :],
                                  in_=xt8_d.ap()[0, :, kq:kq + 4, :])
            nc.sync.dma_start(out=wk_s, in_=wk_d.ap())
            xtb_0 = xtb_pool.tile([128, KT, TCH], bf16, tag="xtb")
            nc.sync.dma_start(out=xtb_0, in_=xtb_d.ap()[0])
            nc.sync.dma_start(out=wv_s, in_=wv_d.ap())
            xt8_1 = xt8_pool.tile([128, KT, TCH], fp8, tag="xt8")
            nc.sync.dma_start(out=xt8_1, in_=xt8_d.ap()[1])
            xtb_1 = xtb_pool.tile([128, KT, TCH], bf16, tag="xtb")
            nc.sync.dma_start(out=xtb_1, in_=xtb_d.ap()[1])
            cos_s = consts.tile([128, T], bf16, tag="cos")
            sin_s = consts.tile([128, T], bf16, tag="sin")
            nc.sync.dma_start(out=cos_s, in_=cos_d.ap())
            nc.sync.dma_start(out=sin_s, in_=sin_d.ap())
            tri_s = consts.tile([128, 128], bf16, tag="tri")
            ident = consts.tile([128, 128], bf16, tag="ident")
            make_identity(nc, ident[:])

            # ---- persistent activations
            qt_s = persist.tile([128, 2, NT], bf16, tag="qt")  # [d, head, tok]
            kt_s = persist.tile([128, 2, NT], bf16, tag="kt")
            v_s = persist.tile([128, NST, 2, 132], bf16, tag="v")
            nc.vector.memset(v_s[:, :, :, 128:129], 1.0)

            # ================= phase 1: projections + rope =================
            for ch in range(NCH):
                t0 = ch * TCH
                pos0 = t0 % T          # position within batch
                if ch == 0:
                    xt8_s, xtb_s = xt8_0, xtb_0
                elif ch == 1:
                    xt8_s, xtb_s = xt8_1, xtb_1
                else:
                    xt8_s = xt8_pool.tile([128, KT, TCH], fp8, tag="xt8")
                    nc.sync.dma_start(out=xt8_s, in_=xt8_d.ap()[ch])
                    xtb_s = xtb_pool.tile([128, KT, TCH], bf16, tag="xtb")
                    nc.sync.dma_start(out=xtb_s, in_=xtb_d.ap()[ch])
                if ch == 2:
                    # attention-phase constants, behind the hot early DMAs
                    nc.sync.dma_start(out=tri_s, in_=tri_d.ap())
                    nc.sync.dma_start(
                        out=wo_s, in_=wo_d.ap().rearrange("k p n -> p k n"))

                # q/k projections: fp8 DoubleRow (channel-major out) + rope
                for w_s, dst in ((wq_s, qt_s), (wk_s, kt_s)):
                    for a in range(2):  # head within component
                        ps_q = psbig.tile([128, TCH], f32, tag="big")
                        for kp in range(KT // 2):
                            nc.tensor.matmul(
                                ps_q[:],
                                lhsT=w_s[:, 2 * kp:2 * kp + 2,
                                         a * 128:(a + 1) * 128],
                                rhs=xt8_s[:, 2 * kp:2 * kp + 2, :],
                                start=(kp == 0), stop=(kp == KT // 2 - 1),
                                perf_mode=DR)
                        # rope: out = q * cos_dup + swap(q) * sin_signed
                        # (swap of partition halves must go through DMA --
                        # compute engines cannot move data across partitions)
                        q_sb = rope_pool.tile([128, TCH], bf16, tag="qsb")
                        nc.scalar.copy(out=q_sb[:], in_=ps_q[:])
                        qsw = rope_pool.tile([128, TCH], bf16, tag="qsw")
                        nc.sync.dma_start(out=qsw[0:64, :], in_=q_sb[64:128, :])
                        nc.sync.dma_start(out=qsw[64:128, :], in_=q_sb[0:64, :])
                        t1 = rope_pool.tile([128, TCH], bf16, tag="t1")
                        nc.vector.tensor_mul(
                            t1[:], q_sb[:], cos_s[:, pos0:pos0 + TCH])
                        t2 = rope_pool.tile([128, TCH], bf16, tag="t2")
                        nc.vector.tensor_mul(
                            t2[:], qsw[:], sin_s[:, pos0:pos0 + TCH])
                        nc.vector.tensor_add(
                            dst[:, a, t0:t0 + TCH], t1[:], t2[:])

                # v projection in bf16 (natural layout); accuracy-critical,
                # so no fp8 on this path
                for st in range(TCH // 128):
                    stg = t0 // 128 + st
                    ps_v = psv.tile([128, P], f32, tag="psv")
                    for k in range(KT):
                        nc.tensor.matmul(
                            ps_v[:],
                            lhsT=xtb_s[:, k, st * 128:(st + 1) * 128],
                            rhs=wv_s[:, k, :],
                            start=(k == 0), stop=(k == KT - 1))
                    # [t, (head d)] -> v_s[:, stg, head, 0:128]
                    nc.vector.tensor_copy(
                        v_s[:, stg, :, 0:128],
                        ps_v[:].rearrange("p (a d) -> p a d", a=2))

            # ====== phases 2+3: causal attention + partial o-proj, per batch
            ystages = {}

            def oproj_chunk(b, lch):
                # partial o-proj for one 512-token chunk of batch b:
                # outT_partial[cout, t] = sum_{cin in c} Wo[cin,cout] yT[cin,t]
                lt0 = lch * TCH
                t0 = b * T + lt0
                for m in range(C // 128):            # 16 cout tiles
                    ps = psbig.tile([128, 512], f32, tag="big")
                    for k in range(2):               # cin k-tiles (= heads)
                        nc.tensor.matmul(
                            ps[:, 0:TCH],
                            lhsT=wo_s[:, k, m * 128:(m + 1) * 128],
                            rhs=ystages[(b, k)][:, lt0:lt0 + TCH],
                            start=(k == 0), stop=(k == 1))
                    # copies split 3:1 DVE/ACT -- ACT also carries the
                    # exp stream of the interleaved attention phase
                    o_sb = o_pool.tile([128, TCH], bf16, tag="osb")
                    if m % 4 != 3:
                        nc.vector.tensor_copy(o_sb[:], ps[:, 0:TCH])
                    else:
                        nc.scalar.copy(out=o_sb[:], in_=ps[:, 0:TCH])
                    nc.sync.dma_start(
                        out=out_d.ap()[m * 128:(m + 1) * 128,
                                       t0:t0 + TCH],
                        in_=o_sb[:])

            for b in range(B):
                for a in range(2):
                    qh = qt_s[:, a, b * T:(b + 1) * T]   # [128, T]
                    kh = kt_s[:, a, b * T:(b + 1) * T]
                    # --- A: scores^T + exp per s-tile (triangular tiles:
                    # pt_j holds columns q in [128j, T)).  j=0/1 are
                    # double-buffered: they are the widest tiles and the
                    # last ones the previous head's PV releases.
                    pts = [None] * ST
                    for j in range(ST):
                        wj = T - 128 * j
                        pt_j = pt_pool.tile([128, wj], bf16, tag=f"pt{j}",
                                            bufs=2 if j < 2 else 1)
                        pts[j] = pt_j
                        q0 = 128 * j
                        while q0 < T:
                            w = min(512, T - q0)
                            ps = psbig.tile([128, 512], f32, tag="big")
                            nc.tensor.matmul(
                                ps[:, 0:w],
                                lhsT=kh[:, 128 * j:128 * (j + 1)],
                                rhs=qh[:, q0:q0 + w],
                                start=True, stop=True)
                            # exp with the folded s_q*s_k/sqrt(HD) scale
                            nc.scalar.activation(
                                out=pt_j[:, q0 - 128 * j:q0 - 128 * j + w],
                                in_=ps[:, 0:w],
                                func=mybir.ActivationFunctionType.Exp,
                                scale=alpha_s[:, 0:1])
                            q0 += w
                        # causal mask on the diagonal 128x128 block; on
                        # GpSimd -- the idle engine -- so the DVE/ACT
                        # stay off the PV critical path
                        nc.gpsimd.tensor_mul(
                            pt_j[:, 0:128], pt_j[:, 0:128], tri_s[:])
                    # --- B: y = P^T.T @ [v|1], normalize, transpose.
                    # Per 4-tile chunk; after head 1 finishes chunk c, the
                    # o-proj for token chunk c of this batch is emitted
                    # immediately (it needs only y tiles 4c..4c+3 of both
                    # heads), spreading its matmuls and 8MB of stores into
                    # the attention phase.  The very last (b=1, a=1) pass
                    # runs chunks ascending so the kernel tail ends on the
                    # longest, densest PV chain instead of idle stores.
                    y_stage = ystg_pool.tile([128, T], bf16, tag=f"ys{a}")
                    ystages[(b, a)] = y_stage
                    corder = range(ST // 4) if (b, a) == (1, 1) \
                        else reversed(range(ST // 4))
                    for c in corder:
                        y_sbs = {}
                        for i in reversed(range(4 * c, 4 * c + 4)):
                            psy = ps_y.tile([128, 132], f32, tag="psy")
                            for j in range(i + 1):
                                nc.tensor.matmul(
                                    psy[:, 0:129],
                                    lhsT=pts[j][:, 128 * (i - j):
                                                128 * (i - j) + 128],
                                    rhs=v_s[:, b * ST + j, a, 0:129],
                                    start=(j == 0), stop=(j == i))
                            recip = small_pool.tile([128, 1], f32,
                                                    tag="recip")
                            nc.vector.reciprocal(recip[:], psy[:, 128:129])
                            # normalize: alternate DVE/ACT so neither
                            # engine serializes the PV chain (ACT reads
                            # f32 PSUM with a per-partition scale AP)
                            y_sb = ysb_pool.tile([128, 128], bf16,
                                                 tag="ysb")
                            if i % 2 == 0:
                                nc.vector.tensor_scalar_mul(
                                    y_sb[:], psy[:, 0:128], recip[:])
                            else:
                                nc.scalar.activation(
                                    out=y_sb[:], in_=psy[:, 0:128],
                                    func=mybir.ActivationFunctionType.Copy,
                                    scale=recip[:, 0:1])
                            y_sbs[i] = y_sb
                        for i in reversed(range(4 * c, 4 * c + 4)):
                            pst = ps_t.tile([128, 128], bf16, tag="pst")
                            nc.tensor.transpose(pst[:], y_sbs[i][:],
                                                ident[:])
                            # NB: must stay on DVE -- ACT reading bf16
                            # PSUM hard-faulted the exec unit on HW
                            nc.vector.tensor_copy(
                                y_stage[:, 128 * i:128 * (i + 1)], pst[:])
                        if a == 1:
                            oproj_chunk(b, c)

    nc.compile()
    return nc


# ------------------------------------------------------------------ entrypoint
_NC_CACHE: dict = {}


def _get_nc(T: int):
    if T not in _NC_CACHE:
        _NC_CACHE[T] = build_nc(T)
    return _NC_CACHE[T]


def assemble_output(results: list[dict], T: int = T_FULL,
                    out_scale: float = 1.0) -> np.ndarray:
    # unshard = sum of the 8 tensor-parallel partial projections (bf16 -> f32)
    outT = results[0]["outt"].astype(np.float32)                # [C, NT]
    for r in results[1:]:
        outT += r["outt"].astype(np.float32)
    outT *= np.float32(out_scale)
    return np.ascontiguousarray(outT.T).reshape(B, T, C).astype(np.float32)


def kernel(**inputs) -> np.ndarray:
    nc = _get_nc(T_FULL)
    in_maps, out_scale = prep_inputs(inputs, T_FULL)
    res = run_bass_kernel_spmd(nc, in_maps, list(range(N_CORES)))
    return assemble_output(res.results, T_FULL, out_scale)


# revision 19
# speedup vs baseline: 1.1647x; 1.1647x over previous
"""Trainium2 Bass kernel for the octonion causal self-attention block.

Strategy (8 NeuronCores, SPMD):
  Each core owns one octonion component c (= heads 2c, 2c+1).
  - Host prep: ternary-quantize weights exactly as the reference does and
    keep them as EXACT {-1,0,+1} matrices (fp8/bf16 representable); the
    scalar scales are folded into (a) the exp() activation scale for
    s_q*s_k/sqrt(HD), and (b) the host-side unshard for s_v*s_o.
    Assemble the effective [C, C] block matrices (sign/permutation
    combine folded in), permute q/k output channels into RoPE
    split-layout, and transpose x to channel-major xT in chunk-
    contiguous layout (one fp8 copy for q/k, one bf16 copy for v).
  - Device phase 0: a short burst of warm-up matmuls on a zero tile so
    the PE HAM clock-gate reaches K=8/8 during the initial DMA wait.
  - Device phase 1: q/k projections in fp8 DoubleRow (2x contraction per
    cycle; ternary weights are exact in fp8, only x carries quantization
    noise which the softmax largely cancels), v projection in bf16.
    RoPE applied on the fly; qT/kT kept channel-major in SBUF.
  - Device phase 2: causal attention per (batch, head): S^T = K^T Q per
    s-tile, exp with the folded score scale (no max subtraction --
    scores are O(1) bounded), triangular mask on the diagonal tile,
    P^T V with an appended ones-column in V giving the softmax
    denominator for free, per-partition normalize, PE-transpose back to
    channel-major yT (kept in SBUF).
  - Device phase 3: PARTIAL output projection with ternary bf16 Wo:
    out_partial^T = Wo[rows of component c]^T @ yT_c -- no collective.
    The host scales by s_v*s_o and sums the 8 partial outputs while
    unsharding.
"""

import numpy as np
import ml_dtypes

import concourse.bass as bass
import concourse.tile as tile
from concourse import bacc, mybir
from concourse.bass_utils import run_bass_kernel_spmd
from concourse.masks import make_identity

# ---------------------------------------------------------------- problem dims
B, T_FULL, C, H = 2, 2048, 2048, 16
HD = C // H          # 128
P = C // 8           # 256
N_CORES = 8
KT = C // 128        # 16 contraction k-tiles

OCT_SIGN = np.array([
    [1, 1, 1, 1, 1, 1, 1, 1],
    [1,-1, 1,-1, 1,-1,-1, 1],
    [1,-1,-1, 1, 1, 1,-1,-1],
    [1, 1,-1,-1, 1,-1, 1,-1],
    [1,-1,-1,-1,-1, 1, 1, 1],
    [1, 1,-1, 1,-1,-1,-1, 1],
    [1, 1, 1,-1,-1, 1,-1,-1],
    [1,-1, 1, 1,-1,-1, 1,-1]], dtype=np.float32)
OCT_IDX = np.array([
    [0,1,2,3,4,5,6,7],
    [1,0,3,2,5,4,7,6],
    [2,3,0,1,6,7,4,5],
    [3,2,1,0,7,6,5,4],
    [4,5,6,7,0,1,2,3],
    [5,4,7,6,1,0,3,2],
    [6,7,4,5,2,3,0,1],
    [7,6,5,4,3,2,1,0]], dtype=np.int32)
_COMB = np.zeros((8, 8, 8), np.float32)
for _i in range(8):
    for _j in range(8):
        _COMB[OCT_IDX[_i, _j], _i, _j] = OCT_SIGN[_i, _j]

BF16 = ml_dtypes.bfloat16
FP8 = ml_dtypes.float8_e4m3


# ------------------------------------------------------------------- host prep
def _ternary(W: np.ndarray) -> tuple[np.ndarray, np.float32]:
    """Reference's ternary quantization, returned as exact {-1,0,1} + scale."""
    try:
        import jax
        import jax.numpy as jnp
        cpu = jax.local_devices(backend="cpu")[0]
        with jax.default_device(cpu):
            Wj = jnp.asarray(W)
            s = jnp.mean(jnp.abs(Wj)) + 1e-8
            t = jnp.round(jnp.clip(Wj / s, -1.0, 1.0))
            return np.asarray(t, np.float32), np.float32(s)
    except Exception:
        s = np.float32(np.mean(np.abs(W.astype(np.float32)))) + np.float32(1e-8)
        return np.rint(np.clip(W / s, -1.0, 1.0)).astype(np.float32), s


def _build_t_eff(W: np.ndarray) -> tuple[np.ndarray, np.float32]:
    """[8,P,P] weights -> exact ternary effective [C, C] + scale."""
    Tq, s = _ternary(W)  # (8, P, P) in {-1,0,1}
    # T_eff[(j,p),(k,q)] = sum_i COMB[k,i,j] * Tq[i,p,q]; exactly one i per (j,k)
    return (np.einsum("kij,ipq->jpkq", _COMB, Tq).reshape(C, C), s)


def _rope_colperm() -> np.ndarray:
    """colperm[new] = old: within each head, [re0..re63 | im0..im63]."""
    perm = np.zeros(C, dtype=np.int64)
    for h in range(H):
        base = h * HD
        for r in range(HD // 2):
            perm[base + r] = base + 2 * r
            perm[base + HD // 2 + r] = base + 2 * r + 1
    return perm


def prep_inputs(inputs: dict, T: int) -> list[dict]:
    """Build the 8 per-core input maps from the full problem inputs."""
    NT = B * T
    TCH = min(512, T)
    NCH = NT // TCH
    x = np.asarray(inputs["x"], np.float32)[:, :T, :]
    cos = np.asarray(inputs["freqs_cos"], np.float32)[:T]   # [T, 64]
    sin = np.asarray(inputs["freqs_sin"], np.float32)[:T]

    tq_eff, s_q = _build_t_eff(np.asarray(inputs["wq"], np.float32))
    tk_eff, s_k = _build_t_eff(np.asarray(inputs["wk"], np.float32))
    tv_eff, s_v = _build_t_eff(np.asarray(inputs["wv"], np.float32))
    to_eff, s_o = _build_t_eff(np.asarray(inputs["wo"], np.float32))

    perm = _rope_colperm()
    tq_eff = tq_eff[:, perm]
    tk_eff = tk_eff[:, perm]

    # xT [C, NT] in chunk-contiguous layout [NCH, 128, KT, TCH]:
    # element [ch, p, k, t] = xT[k*128+p, ch*TCH+t] -> per-partition lines
    # are KT*TCH contiguous elements (one DMA descriptor per partition).
    xt = x.reshape(NT, C).T.reshape(KT, 128, NCH, TCH).transpose(2, 1, 0, 3)
    xt8 = np.ascontiguousarray(xt.astype(FP8))
    xtb = np.ascontiguousarray(xt.astype(BF16))

    # rope tables, duplicated-half layout [128, T]
    cosd = np.empty((128, T), np.float32)
    cosd[0:64] = cos.T
    cosd[64:128] = cos.T
    sind = np.empty((128, T), np.float32)
    sind[0:64] = -sin.T
    sind[64:128] = sin.T
    cosd = cosd.astype(BF16)
    sind = sind.astype(BF16)

    tri = np.triu(np.ones((128, 128), np.float32)).astype(BF16)  # [s,q] s<=q

    # folded score scale for the exp() activation
    alpha = np.full((128, 1), s_q * s_k * (HD ** -0.5), np.float32)
    out_scale = float(s_v) * float(s_o)

    def wblocks(t_eff: np.ndarray, c: int, dt) -> np.ndarray:
        # [C, 256] block -> [128, KT, 256] (partition-major, contiguous lines)
        blk = t_eff[:, c * P:(c + 1) * P].reshape(KT, 128, P).transpose(1, 0, 2)
        return np.ascontiguousarray(blk.astype(dt))

    in_maps = []
    for c in range(N_CORES):
        # o-proj row-block for component c: [256, C] -> [2, 128, C]
        wo_rows = np.ascontiguousarray(
            to_eff[c * P:(c + 1) * P, :].reshape(2, 128, C).astype(BF16))
        in_maps.append({
            "xt8": xt8,
            "xtb": xtb,
            "wq": wblocks(tq_eff, c, FP8),
            "wk": wblocks(tk_eff, c, FP8),
            "wv": wblocks(tv_eff, c, BF16),
            "wo": wo_rows,
            "cosd": cosd,
            "sind": sind,
            "tri": tri,
            "alpha": alpha,
        })
    return in_maps, out_scale


# ------------------------------------------------------------- device program
def build_nc(T: int = T_FULL, n_cores: int = N_CORES):
    NT = B * T
    ST = T // 128            # s-tiles per batch
    NST = NT // 128
    TCH = min(512, T)        # token chunk; must not cross a batch boundary
    NCH = NT // TCH
    bf16 = mybir.dt.bfloat16
    fp8 = mybir.dt.float8e4
    f32 = mybir.dt.float32
    DR = mybir.MatmulPerfMode.DoubleRow

    nc = bacc.Bacc("TRN2", target_bir_lowering=False, debug=False,
                   num_devices=n_cores)

    xt8_d = nc.dram_tensor("xt8", [NCH, 128, KT, TCH], fp8, kind="ExternalInput")
    xtb_d = nc.dram_tensor("xtb", [NCH, 128, KT, TCH], bf16, kind="ExternalInput")
    wq_d = nc.dram_tensor("wq", [128, KT, P], fp8, kind="ExternalInput")
    wk_d = nc.dram_tensor("wk", [128, KT, P], fp8, kind="ExternalInput")
    wv_d = nc.dram_tensor("wv", [128, KT, P], bf16, kind="ExternalInput")
    wo_d = nc.dram_tensor("wo", [2, 128, C], bf16, kind="ExternalInput")
    cos_d = nc.dram_tensor("cosd", [128, T], bf16, kind="ExternalInput")
    sin_d = nc.dram_tensor("sind", [128, T], bf16, kind="ExternalInput")
    tri_d = nc.dram_tensor("tri", [128, 128], bf16, kind="ExternalInput")
    alpha_d = nc.dram_tensor("alpha", [128, 1], f32, kind="ExternalInput")
    out_d = nc.dram_tensor("outt", [C, NT], bf16, kind="ExternalOutput")

    with tile.TileContext(nc) as tc:
        # One flat pool scope; engine queues are FIFO, so EMISSION ORDER is
        # the schedule.  The ACT exp stream (~105us minimum: (N+352)/1.2ns
        # per ACTIVATE) is the kernel's backbone -- everything else is
        # interleaved around it at fine grain, with PSUM pools split so
        # alternating streams never contend for banks:
        #   qk-proj + o-proj share 2 banks, v-proj + PV share 2 (same-tag
        #   ring), scores/exp get 3, transpose 1  ->  exactly 8.
        with (
            tc.tile_pool(name="consts", bufs=1) as consts,
            tc.tile_pool(name="persist", bufs=1) as persist,
            tc.tile_pool(name="xt8s", bufs=2) as xt8_pool,
            tc.tile_pool(name="xtbs", bufs=2) as xtb_pool,
            tc.tile_pool(name="rope", bufs=2) as rope_pool,
            tc.tile_pool(name="pt", bufs=1) as pt_pool,
            tc.tile_pool(name="att_small", bufs=4) as small_pool,
            tc.tile_pool(name="ysb", bufs=ST) as ysb_pool,
            tc.tile_pool(name="ostage", bufs=6) as o_pool,
            tc.tile_pool(name="ps_qo", bufs=2, space="PSUM") as ps_qo,
            tc.tile_pool(name="ps_vp", bufs=2, space="PSUM") as ps_vp,
            tc.tile_pool(name="ps_sc", bufs=3, space="PSUM") as ps_sc,
            tc.tile_pool(name="ps_t", bufs=1, space="PSUM") as ps_t,
        ):
            # ---- PE warm-up: ~3.5us of junk matmuls with no DMA deps so
            # the HAM clock-gate flips to K=8/8 while inputs stream in.
            warm = consts.tile([128, 512], bf16, tag="warm")
            nc.vector.memset(warm[:], 0.0)
            ps_w = ps_sc.tile([128, 512], f32, tag="sc")
            for _ in range(8):
                nc.tensor.matmul(ps_w[:], lhsT=warm[:, 0:128], rhs=warm[:],
                                 start=True, stop=True)

            # ---- resident constants (DMA order = PE consumption order)
            alpha_s = consts.tile([128, 1], f32, tag="alpha")
            nc.sync.dma_start(out=alpha_s, in_=alpha_d.ap())
            wq_s = consts.tile([128, KT, P], fp8, tag="wq")
            wk_s = consts.tile([128, KT, P], fp8, tag="wk")
            wv_s = consts.tile([128, KT, P], bf16, tag="wv")
            wo_s = consts.tile([128, 2, C], bf16, tag="wo")
            xt8_tiles = {}
            xtb_tiles = {}
            xt8_tiles[0] = xt8_pool.tile([128, KT, TCH], fp8, tag="xt8",
                                         name="xt8_0")
            for kq in range(0, KT, 4):
                nc.sync.dma_start(out=wq_s[:, kq:kq + 4, :],
                                  in_=wq_d.ap()[:, kq:kq + 4, :])
                nc.sync.dma_start(out=xt8_tiles[0][:, kq:kq + 4, :],
                                  in_=xt8_d.ap()[0, :, kq:kq + 4, :])
            nc.sync.dma_start(out=wk_s, in_=wk_d.ap())
            xtb_tiles[0] = xtb_pool.tile([128, KT, TCH], bf16, tag="xtb",
                                         name="xtb_0")
            nc.sync.dma_start(out=xtb_tiles[0], in_=xtb_d.ap()[0])
            nc.sync.dma_start(out=wv_s, in_=wv_d.ap())
            xt8_tiles[1] = xt8_pool.tile([128, KT, TCH], fp8, tag="xt8",
                                         name="xt8_1")
            nc.sync.dma_start(out=xt8_tiles[1], in_=xt8_d.ap()[1])
            xtb_tiles[1] = xtb_pool.tile([128, KT, TCH], bf16, tag="xtb",
                                         name="xtb_1")
            nc.sync.dma_start(out=xtb_tiles[1], in_=xtb_d.ap()[1])
            cos_s = consts.tile([128, T], bf16, tag="cos")
            sin_s = consts.tile([128, T], bf16, tag="sin")
            nc.sync.dma_start(out=cos_s, in_=cos_d.ap())
            nc.sync.dma_start(out=sin_s, in_=sin_d.ap())
            tri_s = consts.tile([128, 128], bf16, tag="tri")
            ident = consts.tile([128, 128], bf16, tag="ident")
            make_identity(nc, ident[:])

            # ---- persistent activations
            qt_s = persist.tile([128, 2, NT], bf16, tag="qt")  # [d, head, tok]
            kt_s = persist.tile([128, 2, NT], bf16, tag="kt")
            v_s = persist.tile([128, NST, 2, 132], bf16, tag="v")
            nc.vector.memset(v_s[:, :, :, 128:129], 1.0)
            ystages = {}
            for b in range(B):
                for a in range(2):
                    ystages[(b, a)] = persist.tile(
                        [128, T], bf16, tag=f"ystage{b}{a}",
                        name=f"ystage{b}{a}")

            # ---------------- emission building blocks ----------------
            def prefetch(ch):
                if ch >= NCH or ch in xt8_tiles:
                    return
                xt8_tiles[ch] = xt8_pool.tile([128, KT, TCH], fp8, tag="xt8",
                                              name=f"xt8_{ch}")
                nc.sync.dma_start(out=xt8_tiles[ch], in_=xt8_d.ap()[ch])
                xtb_tiles[ch] = xtb_pool.tile([128, KT, TCH], bf16, tag="xtb",
                                              name=f"xtb_{ch}")
                nc.sync.dma_start(out=xtb_tiles[ch], in_=xtb_d.ap()[ch])

            def qk_group(ch, wi, a):
                # one q-or-k projection group: fp8 DoubleRow + rope
                t0 = ch * TCH
                pos0 = t0 % T
                w_s, dst = ((wq_s, qt_s), (wk_s, kt_s))[wi]
                ps_q = ps_qo.tile([128, TCH], f32, tag="qo", name="ps_q")
                for kp in range(KT // 2):
                    nc.tensor.matmul(
                        ps_q[:],
                        lhsT=w_s[:, 2 * kp:2 * kp + 2, a * 128:(a + 1) * 128],
                        rhs=xt8_tiles[ch][:, 2 * kp:2 * kp + 2, :],
                        start=(kp == 0), stop=(kp == KT // 2 - 1),
                        perf_mode=DR)
                # rope: out = q * cos_dup + swap(q) * sin_signed (the swap of
                # partition halves must go through DMA).  The PSUM copy sits
                # on ACT early on, DVE once the exp stream occupies ACT.
                q_sb = rope_pool.tile([128, TCH], bf16, tag="qsb", name="q_sb")
                if ch < 4:
                    nc.scalar.copy(out=q_sb[:], in_=ps_q[:])
                else:
                    nc.vector.tensor_copy(q_sb[:], ps_q[:])
                qsw = rope_pool.tile([128, TCH], bf16, tag="qsw", name="qsw")
                nc.sync.dma_start(out=qsw[0:64, :], in_=q_sb[64:128, :])
                nc.sync.dma_start(out=qsw[64:128, :], in_=q_sb[0:64, :])
                t1 = rope_pool.tile([128, TCH], bf16, tag="t1", name="t1")
                nc.vector.tensor_mul(t1[:], q_sb[:], cos_s[:, pos0:pos0 + TCH])
                t2 = rope_pool.tile([128, TCH], bf16, tag="t2", name="t2")
                nc.vector.tensor_mul(t2[:], qsw[:], sin_s[:, pos0:pos0 + TCH])
                nc.vector.tensor_add(dst[:, a, t0:t0 + TCH], t1[:], t2[:])

            def v_group(ch, st):
                # one v-projection group, bf16 (accuracy-critical path)
                t0 = ch * TCH
                stg = t0 // 128 + st
                ps_v = ps_vp.tile([128, P], f32, tag="vp", name="ps_v")
                for k in range(KT):
                    nc.tensor.matmul(
                        ps_v[:, 0:P],
                        lhsT=xtb_tiles[ch][:, k, st * 128:(st + 1) * 128],
                        rhs=wv_s[:, k, :],
                        start=(k == 0), stop=(k == KT - 1))
                nc.vector.tensor_copy(
                    v_s[:, stg, :, 0:128],
                    ps_v[:, 0:P].rearrange("p (a d) -> p a d", a=2))

            def proj_chunk(ch):
                for wi in range(2):
                    for a in range(2):
                        qk_group(ch, wi, a)
                for st in range(TCH // 128):
                    v_group(ch, st)

            def scores_j(b, a, j, pts):
                # scores^T + exp for s-tile j (pt_j holds cols q in [128j, T))
                qh = qt_s[:, a, b * T:(b + 1) * T]
                kh = kt_s[:, a, b * T:(b + 1) * T]
                wj = T - 128 * j
                pt_j = pt_pool.tile([128, wj], bf16, tag=f"pt{j}",
                                    name=f"pt{j}")
                pts[j] = pt_j
                q0 = 128 * j
                while q0 < T:
                    w = min(512, T - q0)
                    ps = ps_sc.tile([128, 512], f32, tag="sc", name="ps_sc")
                    nc.tensor.matmul(
                        ps[:, 0:w],
                        lhsT=kh[:, 128 * j:128 * (j + 1)],
                        rhs=qh[:, q0:q0 + w],
                        start=True, stop=True)
                    # exp with the folded s_q*s_k/sqrt(HD) scale
                    nc.scalar.activation(
                        out=pt_j[:, q0 - 128 * j:q0 - 128 * j + w],
                        in_=ps[:, 0:w],
                        func=mybir.ActivationFunctionType.Exp,
                        scale=alpha_s[:, 0:1])
                    q0 += w
                # causal mask on the diagonal 128x128 block, on the otherwise
                # idle GpSimd so DVE/ACT stay off the PV critical path
                nc.gpsimd.tensor_mul(pt_j[:, 0:128], pt_j[:, 0:128], tri_s[:])

            def pv_chunk(b, a, c, pts):
                # y = P^T.T @ [v|1] for q-tiles 4c..4c+3, normalize, transpose
                y_stage = ystages[(b, a)]
                y_sbs = {}
                for i in reversed(range(4 * c, 4 * c + 4)):
                    psy = ps_vp.tile([128, 132], f32, tag="vp", name="psy")
                    for j in range(i + 1):
                        nc.tensor.matmul(
                            psy[:, 0:129],
                            lhsT=pts[j][:, 128 * (i - j):128 * (i - j) + 128],
                            rhs=v_s[:, b * ST + j, a, 0:129],
                            start=(j == 0), stop=(j == i))
                    recip = small_pool.tile([128, 1], f32, tag="recip",
                                            name="recip")
                    nc.vector.reciprocal(recip[:], psy[:, 128:129])
                    # normalize: alternate DVE/ACT so neither engine
                    # serializes the PV chain
                    y_sb = ysb_pool.tile([128, 128], bf16, tag="ysb",
                                         name="y_sb")
                    if i % 2 == 0:
                        nc.vector.tensor_scalar_mul(
                            y_sb[:], psy[:, 0:128], recip[:])
                    else:
                        nc.scalar.activation(
                            out=y_sb[:], in_=psy[:, 0:128],
                            func=mybir.ActivationFunctionType.Copy,
                            scale=recip[:, 0:1])
                    y_sbs[i] = y_sb
                for i in reversed(range(4 * c, 4 * c + 4)):
                    pst = ps_t.tile([128, 128], bf16, tag="pst", name="pst")
                    nc.tensor.transpose(pst[:], y_sbs[i][:], ident[:])
                    # NB: must stay on DVE -- ACT reading bf16 PSUM
                    # hard-faulted the exec unit on HW
                    nc.vector.tensor_copy(
                        y_stage[:, 128 * i:128 * (i + 1)], pst[:])

            def oproj_chunk(b, lch):
                # partial o-proj for one 512-token chunk of batch b
                lt0 = lch * TCH
                t0 = b * T + lt0
                for m in range(C // 128):            # 16 cout tiles
                    ps = ps_qo.tile([128, 512], f32, tag="qo", name="ps_o")
                    for k in range(2):               # cin k-tiles (= heads)
                        nc.tensor.matmul(
                            ps[:, 0:TCH],
                            lhsT=wo_s[:, k, m * 128:(m + 1) * 128],
                            rhs=ystages[(b, k)][:, lt0:lt0 + TCH],
                            start=(k == 0), stop=(k == 1))
                    # copies split 3:1 DVE/ACT -- ACT carries the exp stream
                    o_sb = o_pool.tile([128, TCH], bf16, tag="osb",
                                       name="o_sb")
                    if m % 4 != 3:
                        nc.vector.tensor_copy(o_sb[:], ps[:, 0:TCH])
                    else:
                        nc.scalar.copy(out=o_sb[:], in_=ps[:, 0:TCH])
                    nc.sync.dma_start(
                        out=out_d.ap()[m * 128:(m + 1) * 128, t0:t0 + TCH],
                        in_=o_sb[:])

            # ---------------- emission schedule ----------------
            # proj chunks 0-3 back to back (b0 attention needs them all)
            for ch in range(4):
                prefetch(ch + 1)
                if ch == 2:
                    nc.sync.dma_start(out=tri_s, in_=tri_d.ap())
                    nc.sync.dma_start(
                        out=wo_s, in_=wo_d.ap().rearrange("k p n -> p k n"))
                proj_chunk(ch)

            pts00 = [None] * ST
            pts01 = [None] * ST
            pts10 = [None] * ST
            pts11 = [None] * ST

            # b0/a0 scores x ch4, PV x ch5; b0/a1 scores x ch6, PV x ch7.
            # Each interleave pairs an exp-paced block (scores) or a
            # DVE-paced block (PV) with dense proj matmuls from a disjoint
            # PSUM pool, so the PE never head-of-line blocks for long.
            prefetch(5)
            for j in range(ST):
                scores_j(0, 0, j, pts00)
                if j % 4 == 3:
                    qk_group(4, (j // 8) % 2, (j // 4) % 2)
            for st in range(4):
                v_group(4, st)
            prefetch(6)
            for c in reversed(range(4)):
                pv_chunk(0, 0, c, pts00)
                qk_group(5, (3 - c) // 2, (3 - c) % 2)
            for st in range(4):
                v_group(5, st)
            prefetch(7)
            for j in range(ST):
                scores_j(0, 1, j, pts01)
                if j % 4 == 3:
                    qk_group(6, (j // 8) % 2, (j // 4) % 2)
            for st in range(4):
                v_group(6, st)
            for c in reversed(range(4)):
                pv_chunk(0, 1, c, pts01)
                qk_group(7, (3 - c) // 2, (3 - c) % 2)
            for st in range(4):
                v_group(7, st)

            # b1/a0 scores, with b0's o-proj chunks 3,2 as PE filler
            for j in range(ST):
                scores_j(1, 0, j, pts10)
                if j == 3:
                    oproj_chunk(0, 3)
                elif j == 9:
                    oproj_chunk(0, 2)
            for c in reversed(range(4)):
                pv_chunk(1, 0, c, pts10)
            # b1/a1 scores, with b0's remaining o-proj chunks as filler
            for j in range(ST):
                scores_j(1, 1, j, pts11)
                if j == 3:
                    oproj_chunk(0, 1)
                elif j == 9:
                    oproj_chunk(0, 0)
            # final PV ascending + per-chunk o-proj: the kernel tail ends on
            # the longest, densest PV chain instead of idle stores
            for c in range(4):
                pv_chunk(1, 1, c, pts11)
                oproj_chunk(1, c)

    nc.compile()
    return nc


# ------------------------------------------------------------------ entrypoint
_NC_CACHE: dict = {}


def _get_nc(T: int):
    if T not in _NC_CACHE:
        _NC_CACHE[T] = build_nc(T)
    return _NC_CACHE[T]


def assemble_output(results: list[dict], T: int = T_FULL,
                    out_scale: float = 1.0) -> np.ndarray:
    # unshard = sum of the 8 tensor-parallel partial projections (bf16 -> f32)
    outT = results[0]["outt"].astype(np.float32)                # [C, NT]
    for r in results[1:]:
        outT += r["outt"].astype(np.float32)
    outT *= np.float32(out_scale)
    return np.ascontiguousarray(outT.T).reshape(B, T, C).astype(np.float32)


def kernel(**inputs) -> np.ndarray:
    nc = _get_nc(T_FULL)
    in_maps, out_scale = prep_inputs(inputs, T_FULL)
    res = run_bass_kernel_spmd(nc, in_maps, list(range(N_CORES)))
    return assemble_output(res.results, T_FULL, out_scale)
